# revision 7
# baseline (speedup 1.0000x reference)
"""Bass/Trainium2 kernel for nn_BDHAttentionLayer (B=2, S=2048, DM=1024, H=16).

ReLU-attention layer: Q/K/V projections, causal relu-normalized attention,
output projection. Sharded over 8 NeuronCores: data-parallel over batch (2)
x head-parallel (16 heads -> 4 heads per core). Each core computes a partial
(DM, S) transposed output for its batch; host sums the 4 head-group partials
per batch, transposes, and adds the output bias.

Fully fused single-pass structure per 512-wide query chunk j:
  x-load(j) -> Q/K proj(j) -> V proj(k-tiles 4j..4j+3) -> attention(j, both
  head pairs) -> normalization -> output projection(j), software-pipelined
  across j so the PE almost never waits on DMA or evacuations.

Normalization (attn row-sums via a ones-column appended to V, landing at
PSUM partition 64) computes 1/(den+eps) as exp(-ln(den+eps)) on the Scalar
engine (both funcs live in one act table with Relu/Identity/Copy so there is
no table thrashing), broadcasts it across partitions with a 0-stride-source
DMA, and applies it with one DVE multiply per head pair.

Relu evacuations of the score PSUM tiles rotate across DVE / Act / Pool so
no single engine bottlenecks; V-bias adds run on Pool; Q/K bias-evacuations
on Act/DVE; output tiles DMA straight from PSUM to HBM.

All matmuls run in float32r (full-rate fp32 PE mode). PSUM evacuations read
PSUM through a float32r bitcast so the writes stay on the fast same-dtype
path while still satisfying the FP32r producer check.
"""

import contextlib

import numpy as np

import concourse.bass as bass
import concourse.mybir as mybir
import concourse.tile as tile
from concourse import bacc
from concourse.bass import ds, ts
from concourse.bass_utils import run_bass_kernel_spmd

B, S, DM, H = 2, 2048, 1024, 16
DH = DM // H
EPS = 1e-9
N_CORES = 8
HPC = H // (N_CORES // B)  # heads per core = 4
DCG = HPC * DH  # hidden dims per core = 256
KO = DM // 128  # contraction tiles for projections = 8
SC = S // 512  # 512-wide q chunks = 4
ST = S // 128  # 128-wide s/k tiles = 16
VW = DH + 1  # V tile width: 64 ctx dims + ones col = 65
SCALE = 1.0 / np.sqrt(DH)

F32 = mybir.dt.float32
F32R = mybir.dt.float32r
AF = mybir.ActivationFunctionType

_CACHED = {}


def _build(reps=1, phases=3):
    nc = bacc.Bacc("TRN2", debug=False, num_devices=N_CORES)
    xT = nc.dram_tensor("xT", (DM, S), F32R, kind="ExternalInput")
    wq = nc.dram_tensor("wq", (DM, DCG), F32R, kind="ExternalInput")
    wk = nc.dram_tensor("wk", (DM, DCG), F32R, kind="ExternalInput")
    wv = nc.dram_tensor("wv", (DM, DCG), F32R, kind="ExternalInput")
    wo = nc.dram_tensor("wo", (DCG, DM), F32R, kind="ExternalInput")
    bqv = nc.dram_tensor("bqv", (DCG, 1), F32, kind="ExternalInput")
    bkv = nc.dram_tensor("bkv", (DCG, 1), F32, kind="ExternalInput")
    bvv = nc.dram_tensor("bvv", (DCG,), F32R, kind="ExternalInput")
    outT = nc.dram_tensor("outT", (DM, S), F32, kind="ExternalOutput")

    with tile.TileContext(nc) as tc:
        with (
            tc.tile_pool(name="consts", bufs=1) as consts,
            tc.tile_pool(name="qkv", bufs=1) as qkv,
        ):
            # ---- constant loads (outside the rep loop) ----
            wq_sb = consts.tile([128, KO, DCG], F32R)
            wk_sb = consts.tile([128, KO, DCG], F32R)
            wv_sb = consts.tile([128, KO, DCG], F32R)
            wo_sb = consts.tile([128, 2, DM], F32R)
            for ko in range(KO):
                nc.sync.dma_start(out=wq_sb[:, ko, :], in_=wq.ap()[ts(ko, 128), :])
                nc.sync.dma_start(out=wk_sb[:, ko, :], in_=wk.ap()[ts(ko, 128), :])
                nc.sync.dma_start(out=wv_sb[:, ko, :], in_=wv.ap()[ts(ko, 128), :])
            for dc in range(2):
                nc.sync.dma_start(out=wo_sb[:, dc, :], in_=wo.ap()[ts(dc, 128), :])
            bq_sb = consts.tile([128, 2, 1], F32)
            bk_sb = consts.tile([128, 2, 1], F32)
            nc.sync.dma_start(
                out=bq_sb, in_=bqv.ap().rearrange("(t p) o -> p t o", p=128)
            )
            nc.sync.dma_start(
                out=bk_sb, in_=bkv.ap().rearrange("(t p) o -> p t o", p=128)
            )
            bv_bc = consts.tile([128, DCG], F32R)
            bvap = bvv.ap()
            nc.sync.dma_start(
                out=bv_bc,
                in_=bass.AP(tensor=bvap.tensor, offset=0, ap=[[0, 128], [1, DCG]]),
            )


            q_sb = qkv.tile([128, 2, S], F32R)
            k_sb = qkv.tile([128, 2, S], F32R)
            v_sb = qkv.tile([128, ST, HPC, VW], F32R)
            ctx_sb = qkv.tile([128, 2, S], F32R)
            # ones column of V, set once; V-proj writes only cols 0:DH
            nc.vector.memset(v_sb[:, :, :, DH:VW].bitcast(F32), 1.0)

            loop_cm = tc.For_i(0, reps, 1) if reps > 1 else contextlib.nullcontext()
            with loop_cm:
                with (
                    tc.tile_pool(name="xp", bufs=2) as xp,
                    tc.tile_pool(name="ps_big", bufs=2, space="PSUM") as ps_big,
                    tc.tile_pool(name="ps_score", bufs=2, space="PSUM") as ps_score,
                    tc.tile_pool(name="ps_ctx", bufs=2, space="PSUM") as ps_ctx,
                    tc.tile_pool(name="attn", bufs=8) as attn_p,
                    tc.tile_pool(name="smalls", bufs=2) as smalls,
                    tc.tile_pool(name="outs", bufs=3) as outp,
                ):
                    relu_rr = [0]  # relu engine rotation counter

                    def emit_relu(dst, src):
                        # GPSIMD cannot read PSUM on real HW: DVE/Act only
                        eng = relu_rr[0] % 2
                        relu_rr[0] += 1
                        if eng == 0:
                            nc.vector.tensor_scalar_max(dst, src, 0.0)
                        else:
                            nc.scalar.activation(out=dst, in_=src, func=AF.Relu)

                    def load_x(j):
                        jc = ds(512 * j, 512)
                        x_ch = xp.tile([128, KO, 512], F32R, tag="x")
                        for ko in range(KO):
                            # Act's HWDGE queue: keeps the bulk x stream off
                            # the SP queue so norm DMAs can't convoy it
                            nc.scalar.dma_start(
                                out=x_ch[:, ko, :], in_=xT.ap()[ts(ko, 128), jc]
                            )
                        return x_ch

                    def out_proj(j):
                        jc = ds(512 * j, 512)
                        for dt in range(KO):
                            po = ps_big.tile([128, 512], F32, tag="pp")
                            for dc in range(2):
                                nc.tensor.matmul(
                                    po,
                                    wo_sb[:, dc, ts(dt, 128)],
                                    ctx_sb[:, dc, jc],
                                    start=(dc == 0),
                                    stop=(dc == 1),
                                )
                            ot = outp.tile([128, 512], F32, tag="ot")
                            nc.vector.tensor_copy(ot[:, 0:256], po[:, 0:256])
                            nc.scalar.activation(
                                out=ot[:, 256:512],
                                in_=po[:, 256:512],
                                func=AF.Copy,
                            )
                            nc.sync.dma_start(
                                out=outT.ap()[ts(dt, 128), jc], in_=ot
                            )

                    x_tiles = {0: load_x(0)}
                    for j in range(SC):
                        jc = ds(512 * j, 512)
                        if j + 1 < SC:
                            x_tiles[j + 1] = load_x(j + 1)
                        x_ch = x_tiles.pop(j)
                        # ---- Q/K projections for chunk j ----
                        for t in range(2):
                            pq = ps_big.tile([128, 512], F32, tag="pp")
                            for ko in range(KO):
                                nc.tensor.matmul(
                                    pq,
                                    wq_sb[:, ko, ts(t, 128)],
                                    x_ch[:, ko, :],
                                    start=(ko == 0),
                                    stop=(ko == KO - 1),
                                )
                            nc.scalar.activation(
                                out=q_sb[:, t, jc],
                                in_=pq[:].bitcast(F32R),
                                func=AF.Identity,
                                bias=bq_sb[:, t, 0:1],
                            )
                            pk = ps_big.tile([128, 512], F32, tag="pp")
                            for ko in range(KO):
                                nc.tensor.matmul(
                                    pk,
                                    wk_sb[:, ko, ts(t, 128)],
                                    x_ch[:, ko, :],
                                    start=(ko == 0),
                                    stop=(ko == KO - 1),
                                )
                            nc.vector.tensor_scalar_add(
                                k_sb[:, t, jc],
                                pk[:].bitcast(F32R),
                                bk_sb[:, t, 0:1],
                            )
                        # ---- V projection for k-tiles 4j .. 4j+3 ----
                        for sl in range(4):
                            st = 4 * j + sl
                            pv = ps_big.tile([128, DCG], F32, tag="pp")
                            for ko in range(KO):
                                nc.tensor.matmul(
                                    pv,
                                    x_ch[:, ko, ds(128 * sl, 128)],
                                    wv_sb[:, ko, :],
                                    start=(ko == 0),
                                    stop=(ko == KO - 1),
                                )
                            nc.vector.tensor_add(
                                out=v_sb[:, st, :, 0:DH],
                                in0=pv[:]
                                .bitcast(F32R)
                                .rearrange("p (h d) -> p h d", h=HPC),
                                in1=bv_bc[:].rearrange("p (h d) -> p h d", h=HPC),
                            )

                        if phases < 2:
                            continue
                        # ---- deferred output projection for chunk j-1 ----
                        # (its ctx relocations complete during proj(j) above)
                        if phases >= 3 and j > 0:
                            out_proj(j - 1)
                        # ---- attention for chunk j, both head pairs ----
                        n_k = 4 * j + 4  # causal: k-tiles 0 .. 4j+3
                        norm_state = []
                        for p in range(2):
                            cpair = ps_ctx.tile([VW, 2, 512], F32, tag="ctx")
                            pend = []  # ctx MMs lag two i (sw pipeline)
                            for i in range(n_k):
                                cur = []
                                c0 = 128 * (i - 4 * j) if i >= 4 * j else 0
                                for hh in range(2):
                                    h = 2 * p + hh
                                    base = 64 * hh
                                    sps = ps_score.tile(
                                        [128, 512], F32, tag="score"
                                    )
                                    nc.tensor.matmul(
                                        sps[:, c0:512],
                                        k_sb[base : base + 64, p, ts(i, 128)],
                                        q_sb[
                                            base : base + 64,
                                            p,
                                            ds(512 * j + c0, 512 - c0),
                                        ],
                                        start=True,
                                        stop=True,
                                    )
                                    at = attn_p.tile([128, 512], F32R, tag="attn")
                                    # split the relu across two engines so the
                                    # score PSUM bank frees up ~2x sooner
                                    mid = c0 + 128 if i >= 4 * j else 256
                                    emit_relu(
                                        at[:, c0:mid],
                                        sps[:, c0:mid].bitcast(F32R),
                                    )
                                    if mid < 512:
                                        emit_relu(
                                            at[:, mid:512],
                                            sps[:, mid:512].bitcast(F32R),
                                        )
                                    if i >= 4 * j:  # diagonal 128-col block
                                        # keep where q >= k:
                                        # (512j + f) - (128i + p) >= 0
                                        nc.gpsimd.affine_select(
                                            out=at[:, c0 : c0 + 128],
                                            in_=at[:, c0 : c0 + 128],
                                            compare_op=mybir.AluOpType.is_ge,
                                            fill=0.0,
                                            base=512 * j + c0 - 128 * i,
                                            channel_multiplier=-1,
                                            pattern=[[1, 128]],
                                        )
                                    cur.append((hh, i, at, c0))
                                pend.append(cur)
                                if len(pend) > 2:
                                    for (hh, ii, at, cc) in pend.pop(0):
                                        nc.tensor.matmul(
                                            cpair[0:VW, hh, cc:512],
                                            v_sb[:, ii, 2 * p + hh, :],
                                            at[:, cc:512],
                                            start=(ii == 0),
                                            stop=(ii == n_k - 1),
                                        )
                            for round_ in pend:
                                for (hh, ii, at, cc) in round_:
                                    nc.tensor.matmul(
                                        cpair[0:VW, hh, cc:512],
                                        v_sb[:, ii, 2 * p + hh, :],
                                        at[:, cc:512],
                                        start=(ii == 0),
                                        stop=(ii == n_k - 1),
                                    )
                            # ---- normalization "pre" for pair (p, j):
                            # den row (p64) -> SBUF with +eps, fast recip in
                            # place, one single-descriptor DMA to partition 0
                            den_s = smalls.tile([VW, 2, 512], F32, tag="dens")
                            nc.scalar.activation(
                                out=den_s[64:65, :, :],
                                in_=cpair[64:65, :, :],
                                func=AF.Copy,
                                bias=EPS,
                            )
                            den0 = smalls.tile([1, 2, 512], F32, tag="den0")
                            nc.sync.dma_start(out=den0, in_=den_s[64:65, :, :])
                            # custom-DVE op at partition 0 (p64 breaks on HW)
                            dinv0 = smalls.tile([1, 2, 512], F32, tag="di0")
                            nc.vector.reciprocal_approx_fast(
                                out=dinv0, in_=den0
                            )
                            norm_state.append((cpair, dinv0))

                        # ---- normalization "post" after both pairs' matmuls
                        # so no engine stream blocks mid-attention ----
                        for p, (cpair, dinv0) in enumerate(norm_state):
                            # partition-broadcast 1/den on the Pool engine
                            # (SBUF-only; no PE, no PSUM, no DRAM bounce)
                            bc = smalls.tile([64, 2, 512], F32, tag="bc")
                            nc.gpsimd.partition_broadcast(bc, dinv0)
                            # even head: lanes line up, write ctx_sb directly
                            nc.vector.tensor_mul(
                                out=ctx_sb[0:64, p, jc],
                                in0=cpair[0:64, 0, :].bitcast(F32R),
                                in1=bc[:, 0, :].bitcast(F32R),
                            )
                            # odd head: needs a cross-partition relocation
                            stage = smalls.tile([64, 512], F32R, tag="stage")
                            nc.vector.tensor_mul(
                                out=stage,
                                in0=cpair[0:64, 1, :].bitcast(F32R),
                                in1=bc[:, 1, :].bitcast(F32R),
                            )
                            nc.sync.dma_start(
                                out=ctx_sb[64:128, p, jc], in_=stage
                            )

                    # ---- final chunk's output projection ----
                    if phases >= 3:
                        out_proj(SC - 1)
    nc.compile()
    return nc


def _get_nc():
    if "nc" not in _CACHED:
        _CACHED["nc"] = _build()
    return _CACHED["nc"]


def _in_maps(x, Wq, bq, Wk, bk, Wv, bv, Wo):
    xTs = [np.ascontiguousarray(x[b].T) for b in range(B)]
    maps = []
    for c in range(N_CORES):
        b, hg = divmod(c, N_CORES // B)
        hs = slice(hg * DCG, (hg + 1) * DCG)
        maps.append(
            {
                "xT": xTs[b],
                # fold the 1/sqrt(DH) score scale into the Q projection
                "wq": np.ascontiguousarray(Wq[hs].T) * SCALE,
                "wk": np.ascontiguousarray(Wk[hs].T),
                "wv": np.ascontiguousarray(Wv[hs].T),
                "wo": np.ascontiguousarray(Wo[:, hs].T),
                "bqv": (bq[hs] * SCALE).reshape(DCG, 1).astype(np.float32),
                "bkv": bk[hs].reshape(DCG, 1).astype(np.float32),
                "bvv": bv[hs].astype(np.float32),
            }
        )
    return maps


def kernel(x, Wq, bq, Wk, bk, Wv, bv, Wo, bo, _trace=False):
    x = np.asarray(x, dtype=np.float32)
    Wq, bq = np.asarray(Wq, np.float32), np.asarray(bq, np.float32)
    Wk, bk = np.asarray(Wk, np.float32), np.asarray(bk, np.float32)
    Wv, bv = np.asarray(Wv, np.float32), np.asarray(bv, np.float32)
    Wo, bo = np.asarray(Wo, np.float32), np.asarray(bo, np.float32)

    nc = _get_nc()
    res = run_bass_kernel_spmd(
        nc,
        _in_maps(x, Wq, bq, Wk, bk, Wv, bv, Wo),
        core_ids=list(range(N_CORES)),
        trace=_trace,
    )

    out = np.empty((B, S, DM), dtype=np.float32)
    for b in range(B):
        acc = res.results[b * (N_CORES // B)]["outT"].astype(np.float32)
        for g in range(1, N_CORES // B):
            acc = acc + res.results[b * (N_CORES // B) + g]["outT"]
        out[b] = acc.T + bo
    if _trace:
        return out, res
    return out


# revision 8
# speedup vs baseline: 1.7827x; 1.7827x over previous
"""Bass/Trainium2 kernel for nn_BDHAttentionLayer (B=2, S=2048, DM=1024, H=16).

ReLU-attention layer: Q/K/V projections, causal relu-normalized attention,
output projection. Sharded over 8 NeuronCores: data-parallel over batch (2)
x head-parallel (16 heads -> 4 heads per core). Each core computes a partial
(DM, S) transposed output for its batch; host sums the 4 head-group partials
per batch, transposes, and adds the output bias.

Fully fused single-pass structure per 512-wide query chunk j:
  x-load(j) -> Q/K proj(j) -> V proj(k-tiles 4j..4j+3) -> attention(j, both
  head pairs) -> normalization -> output projection(j), software-pipelined
  across j so the PE almost never waits on DMA or evacuations.

Normalization (attn row-sums via a ones-column appended to V, landing at
PSUM partition 64) computes 1/(den+eps) as exp(-ln(den+eps)) on the Scalar
engine (both funcs live in one act table with Relu/Identity/Copy so there is
no table thrashing), broadcasts it across partitions with a 0-stride-source
DMA, and applies it with one DVE multiply per head pair.

Relu evacuations of the score PSUM tiles rotate across DVE / Act / Pool so
no single engine bottlenecks; V-bias adds run on Pool; Q/K bias-evacuations
on Act/DVE; output tiles DMA straight from PSUM to HBM.

All matmuls run in float32r (full-rate fp32 PE mode). PSUM evacuations read
PSUM through a float32r bitcast so the writes stay on the fast same-dtype
path while still satisfying the FP32r producer check.
"""

import contextlib

import numpy as np

import concourse.bass as bass
import concourse.mybir as mybir
import concourse.tile as tile
from concourse import bacc
from concourse.bass import ds, ts
from concourse.bass_utils import run_bass_kernel_spmd

B, S, DM, H = 2, 2048, 1024, 16
DH = DM // H
EPS = 1e-9
N_CORES = 8
HPC = H // (N_CORES // B)  # heads per core = 4
DCG = HPC * DH  # hidden dims per core = 256
KO = DM // 128  # contraction tiles for projections = 8
SC = S // 512  # 512-wide q chunks = 4
ST = S // 128  # 128-wide s/k tiles = 16
VW = DH + 1  # V tile width: 64 ctx dims + ones col = 65
SCALE = 1.0 / np.sqrt(DH)

F32 = mybir.dt.float32
F32R = mybir.dt.float32r
AF = mybir.ActivationFunctionType

_CACHED = {}


def _build(reps=1, phases=3):
    nc = bacc.Bacc("TRN2", debug=False, num_devices=N_CORES)
    xT = nc.dram_tensor("xT", (DM, S), F32R, kind="ExternalInput")
    wq = nc.dram_tensor("wq", (DM, DCG), F32R, kind="ExternalInput")
    wk = nc.dram_tensor("wk", (DM, DCG), F32R, kind="ExternalInput")
    wv = nc.dram_tensor("wv", (DM, DCG), F32R, kind="ExternalInput")
    wo = nc.dram_tensor("wo", (DCG, DM), F32R, kind="ExternalInput")
    bqv = nc.dram_tensor("bqv", (DCG, 1), F32, kind="ExternalInput")
    bkv = nc.dram_tensor("bkv", (DCG, 1), F32, kind="ExternalInput")
    bvv = nc.dram_tensor("bvv", (DCG,), F32R, kind="ExternalInput")
    outT = nc.dram_tensor("outT", (DM, S), F32, kind="ExternalOutput")

    with tile.TileContext(nc) as tc:
        with (
            tc.tile_pool(name="consts", bufs=1) as consts,
            tc.tile_pool(name="qkv", bufs=1) as qkv,
        ):
            # ---- constant loads (outside the rep loop) ----
            wq_sb = consts.tile([128, KO, DCG], F32R)
            wk_sb = consts.tile([128, KO, DCG], F32R)
            wv_sb = consts.tile([128, KO, DCG], F32R)
            wo_sb = consts.tile([128, 2, DM], F32R)
            for ko in range(KO):
                nc.sync.dma_start(out=wq_sb[:, ko, :], in_=wq.ap()[ts(ko, 128), :])
                nc.sync.dma_start(out=wk_sb[:, ko, :], in_=wk.ap()[ts(ko, 128), :])
                nc.sync.dma_start(out=wv_sb[:, ko, :], in_=wv.ap()[ts(ko, 128), :])
            for dc in range(2):
                nc.sync.dma_start(out=wo_sb[:, dc, :], in_=wo.ap()[ts(dc, 128), :])
            bq_sb = consts.tile([128, 2, 1], F32)
            bk_sb = consts.tile([128, 2, 1], F32)
            nc.sync.dma_start(
                out=bq_sb, in_=bqv.ap().rearrange("(t p) o -> p t o", p=128)
            )
            nc.sync.dma_start(
                out=bk_sb, in_=bkv.ap().rearrange("(t p) o -> p t o", p=128)
            )
            bv_bc = consts.tile([128, DCG], F32R)
            bvap = bvv.ap()
            nc.sync.dma_start(
                out=bv_bc,
                in_=bass.AP(tensor=bvap.tensor, offset=0, ap=[[0, 128], [1, DCG]]),
            )


            q_sb = qkv.tile([128, 2, S], F32R)
            k_sb = qkv.tile([128, 2, S], F32R)
            v_sb = qkv.tile([128, ST, HPC, VW], F32R)
            ctx_sb = qkv.tile([128, 2, S], F32R)
            # ones column of V, set once; V-proj writes only cols 0:DH
            nc.vector.memset(v_sb[:, :, :, DH:VW].bitcast(F32), 1.0)

            loop_cm = tc.For_i(0, reps, 1) if reps > 1 else contextlib.nullcontext()
            with loop_cm:
                with (
                    tc.tile_pool(name="xp", bufs=2) as xp,
                    tc.tile_pool(name="ps_big", bufs=2, space="PSUM") as ps_big,
                    tc.tile_pool(name="ps_score", bufs=2, space="PSUM") as ps_score,
                    tc.tile_pool(name="ps_ctx", bufs=2, space="PSUM") as ps_ctx,
                    tc.tile_pool(name="attn", bufs=8) as attn_p,
                    tc.tile_pool(name="smalls", bufs=2) as smalls,
                    tc.tile_pool(name="outs", bufs=3) as outp,
                ):
                    relu_rr = [0]  # relu engine rotation counter

                    def emit_relu(dst, src):
                        # GPSIMD cannot read PSUM on real HW: DVE/Act only
                        eng = relu_rr[0] % 2
                        relu_rr[0] += 1
                        if eng == 0:
                            nc.vector.tensor_scalar_max(dst, src, 0.0)
                        else:
                            nc.scalar.activation(out=dst, in_=src, func=AF.Relu)

                    def load_x(j):
                        jc = ds(512 * j, 512)
                        x_ch = xp.tile([128, KO, 512], F32R, tag="x")
                        for ko in range(KO):
                            nc.sync.dma_start(
                                out=x_ch[:, ko, :], in_=xT.ap()[ts(ko, 128), jc]
                            )
                        return x_ch

                    def out_proj(j):
                        jc = ds(512 * j, 512)
                        for dt in range(KO):
                            po = ps_big.tile([128, 512], F32, tag="pp")
                            for dc in range(2):
                                nc.tensor.matmul(
                                    po,
                                    wo_sb[:, dc, ts(dt, 128)],
                                    ctx_sb[:, dc, jc],
                                    start=(dc == 0),
                                    stop=(dc == 1),
                                )
                            ot = outp.tile([128, 512], F32, tag="ot")
                            nc.vector.tensor_copy(ot[:, 0:256], po[:, 0:256])
                            nc.scalar.activation(
                                out=ot[:, 256:512],
                                in_=po[:, 256:512],
                                func=AF.Copy,
                            )
                            nc.sync.dma_start(
                                out=outT.ap()[ts(dt, 128), jc], in_=ot
                            )

                    x_tiles = {0: load_x(0)}
                    for j in range(SC):
                        jc = ds(512 * j, 512)
                        if j + 1 < SC:
                            x_tiles[j + 1] = load_x(j + 1)
                        x_ch = x_tiles.pop(j)
                        # ---- Q/K projections for chunk j ----
                        for t in range(2):
                            pq = ps_big.tile([128, 512], F32, tag="pp")
                            for ko in range(KO):
                                nc.tensor.matmul(
                                    pq,
                                    wq_sb[:, ko, ts(t, 128)],
                                    x_ch[:, ko, :],
                                    start=(ko == 0),
                                    stop=(ko == KO - 1),
                                )
                            nc.scalar.activation(
                                out=q_sb[:, t, jc],
                                in_=pq[:].bitcast(F32R),
                                func=AF.Identity,
                                bias=bq_sb[:, t, 0:1],
                            )
                            pk = ps_big.tile([128, 512], F32, tag="pp")
                            for ko in range(KO):
                                nc.tensor.matmul(
                                    pk,
                                    wk_sb[:, ko, ts(t, 128)],
                                    x_ch[:, ko, :],
                                    start=(ko == 0),
                                    stop=(ko == KO - 1),
                                )
                            nc.vector.tensor_scalar_add(
                                k_sb[:, t, jc],
                                pk[:].bitcast(F32R),
                                bk_sb[:, t, 0:1],
                            )
                        # ---- V projection for k-tiles 4j .. 4j+3 ----
                        for sl in range(4):
                            st = 4 * j + sl
                            pv = ps_big.tile([128, DCG], F32, tag="pp")
                            for ko in range(KO):
                                nc.tensor.matmul(
                                    pv,
                                    x_ch[:, ko, ds(128 * sl, 128)],
                                    wv_sb[:, ko, :],
                                    start=(ko == 0),
                                    stop=(ko == KO - 1),
                                )
                            nc.vector.tensor_add(
                                out=v_sb[:, st, :, 0:DH],
                                in0=pv[:]
                                .bitcast(F32R)
                                .rearrange("p (h d) -> p h d", h=HPC),
                                in1=bv_bc[:].rearrange("p (h d) -> p h d", h=HPC),
                            )

                        if phases < 2:
                            continue
                        # ---- deferred output projection for chunk j-1 ----
                        # (its ctx relocations complete during proj(j) above)
                        if phases >= 3 and j > 0:
                            out_proj(j - 1)
                        # ---- attention for chunk j, both head pairs ----
                        n_k = 4 * j + 4  # causal: k-tiles 0 .. 4j+3
                        norm_state = []
                        for p in range(2):
                            cpair = ps_ctx.tile([VW, 2, 512], F32, tag="ctx")
                            pend = []  # ctx MMs lag two i (sw pipeline)
                            for i in range(n_k):
                                cur = []
                                c0 = 128 * (i - 4 * j) if i >= 4 * j else 0
                                for hh in range(2):
                                    h = 2 * p + hh
                                    base = 64 * hh
                                    sps = ps_score.tile(
                                        [128, 512], F32, tag="score"
                                    )
                                    nc.tensor.matmul(
                                        sps[:, c0:512],
                                        k_sb[base : base + 64, p, ts(i, 128)],
                                        q_sb[
                                            base : base + 64,
                                            p,
                                            ds(512 * j + c0, 512 - c0),
                                        ],
                                        start=True,
                                        stop=True,
                                    )
                                    at = attn_p.tile([128, 512], F32R, tag="attn")
                                    # split the relu across two engines so the
                                    # score PSUM bank frees up ~2x sooner
                                    mid = c0 + 128 if i >= 4 * j else 256
                                    emit_relu(
                                        at[:, c0:mid],
                                        sps[:, c0:mid].bitcast(F32R),
                                    )
                                    if mid < 512:
                                        emit_relu(
                                            at[:, mid:512],
                                            sps[:, mid:512].bitcast(F32R),
                                        )
                                    if i >= 4 * j:  # diagonal 128-col block
                                        # keep where q >= k:
                                        # (512j + f) - (128i + p) >= 0
                                        nc.gpsimd.affine_select(
                                            out=at[:, c0 : c0 + 128],
                                            in_=at[:, c0 : c0 + 128],
                                            compare_op=mybir.AluOpType.is_ge,
                                            fill=0.0,
                                            base=512 * j + c0 - 128 * i,
                                            channel_multiplier=-1,
                                            pattern=[[1, 128]],
                                        )
                                    cur.append((hh, i, at, c0))
                                pend.append(cur)
                                if len(pend) > 2:
                                    for (hh, ii, at, cc) in pend.pop(0):
                                        nc.tensor.matmul(
                                            cpair[0:VW, hh, cc:512],
                                            v_sb[:, ii, 2 * p + hh, :],
                                            at[:, cc:512],
                                            start=(ii == 0),
                                            stop=(ii == n_k - 1),
                                        )
                            for round_ in pend:
                                for (hh, ii, at, cc) in round_:
                                    nc.tensor.matmul(
                                        cpair[0:VW, hh, cc:512],
                                        v_sb[:, ii, 2 * p + hh, :],
                                        at[:, cc:512],
                                        start=(ii == 0),
                                        stop=(ii == n_k - 1),
                                    )
                            # ---- normalization "pre" for pair (p, j):
                            # den row (p64) -> SBUF with +eps, fast recip in
                            # place, one single-descriptor DMA to partition 0
                            den_s = smalls.tile([VW, 2, 512], F32, tag="dens")
                            nc.scalar.activation(
                                out=den_s[64:65, :, :],
                                in_=cpair[64:65, :, :],
                                func=AF.Copy,
                                bias=EPS,
                            )
                            den0 = smalls.tile([1, 2, 512], F32, tag="den0")
                            nc.sync.dma_start(out=den0, in_=den_s[64:65, :, :])
                            # custom-DVE op at partition 0 (p64 breaks on HW)
                            dinv0 = smalls.tile([1, 2, 512], F32, tag="di0")
                            nc.vector.reciprocal_approx_fast(
                                out=dinv0, in_=den0
                            )
                            norm_state.append((cpair, dinv0))

                        # ---- normalization "post" after both pairs' matmuls
                        # so no engine stream blocks mid-attention ----
                        for p, (cpair, dinv0) in enumerate(norm_state):
                            # partition-broadcast 1/den on the Pool engine
                            # (SBUF-only; no PE, no PSUM, no DRAM bounce)
                            bc = smalls.tile([64, 2, 512], F32, tag="bc")
                            nc.gpsimd.partition_broadcast(bc, dinv0)
                            # even head: lanes line up, write ctx_sb directly
                            nc.vector.tensor_mul(
                                out=ctx_sb[0:64, p, jc],
                                in0=cpair[0:64, 0, :].bitcast(F32R),
                                in1=bc[:, 0, :].bitcast(F32R),
                            )
                            # odd head: needs a cross-partition relocation
                            stage = smalls.tile([64, 512], F32R, tag="stage")
                            nc.vector.tensor_mul(
                                out=stage,
                                in0=cpair[0:64, 1, :].bitcast(F32R),
                                in1=bc[:, 1, :].bitcast(F32R),
                            )
                            nc.sync.dma_start(
                                out=ctx_sb[64:128, p, jc], in_=stage
                            )

                    # ---- final chunk's output projection ----
                    if phases >= 3:
                        out_proj(SC - 1)
    nc.compile()
    return nc


def _get_nc():
    if "nc" not in _CACHED:
        _CACHED["nc"] = _build()
    return _CACHED["nc"]


def _in_maps(x, Wq, bq, Wk, bk, Wv, bv, Wo):
    xTs = [np.ascontiguousarray(x[b].T) for b in range(B)]
    maps = []
    for c in range(N_CORES):
        b, hg = divmod(c, N_CORES // B)
        hs = slice(hg * DCG, (hg + 1) * DCG)
        maps.append(
            {
                "xT": xTs[b],
                # fold the 1/sqrt(DH) score scale into the Q projection
                "wq": np.ascontiguousarray(Wq[hs].T) * SCALE,
                "wk": np.ascontiguousarray(Wk[hs].T),
                "wv": np.ascontiguousarray(Wv[hs].T),
                "wo": np.ascontiguousarray(Wo[:, hs].T),
                "bqv": (bq[hs] * SCALE).reshape(DCG, 1).astype(np.float32),
                "bkv": bk[hs].reshape(DCG, 1).astype(np.float32),
                "bvv": bv[hs].astype(np.float32),
            }
        )
    return maps


def kernel(x, Wq, bq, Wk, bk, Wv, bv, Wo, bo, _trace=False):
    x = np.asarray(x, dtype=np.float32)
    Wq, bq = np.asarray(Wq, np.float32), np.asarray(bq, np.float32)
    Wk, bk = np.asarray(Wk, np.float32), np.asarray(bk, np.float32)
    Wv, bv = np.asarray(Wv, np.float32), np.asarray(bv, np.float32)
    Wo, bo = np.asarray(Wo, np.float32), np.asarray(bo, np.float32)

    nc = _get_nc()
    res = run_bass_kernel_spmd(
        nc,
        _in_maps(x, Wq, bq, Wk, bk, Wv, bv, Wo),
        core_ids=list(range(N_CORES)),
        trace=_trace,
    )

    out = np.empty((B, S, DM), dtype=np.float32)
    for b in range(B):
        acc = res.results[b * (N_CORES // B)]["outT"].astype(np.float32)
        for g in range(1, N_CORES // B):
            acc = acc + res.results[b * (N_CORES // B) + g]["outT"]
        out[b] = acc.T + bo
    if _trace:
        return out, res
    return out
